# revision 1
# baseline (speedup 1.0000x reference)
"""Additive attention (B=4, Q=256, KV=1024, H=128, VS=256) on 8 Trainium2 cores.

Sharding: each core processes 32 query rows of every batch (4 groups of 32
row-slots).  Per batch, only a KV prefix of width ~valid_len (padded to a
32-multiple with fp32r-legal >=256-wide matmul chunks) is computed; masked
columns beyond it contribute exactly 0 to the softmax, so skipping them is
exact.  No collectives are needed.  The program is specialized per
valid_lens configuration at call time and cached.

Per-core dataflow (groups processed widest-last for fast pipeline start):
  PE    : k/q projections in full fp32 (accuracy-critical, pre-tanh)
  DVE   : sums[h, kv] = kp[h, kv] + qp[h, s]        (tensor_scalar add, 2x)
  ACT   : tanh in place over 8-row batches           (the throughput floor)
  PE    : mask written first via one K=4 matmul (ind outer mask), then score
          rows accumulate via fp32r one-hot matmuls: a [128, 255] window
          tile holds wv only at column 127, so window [127-s : 255-s] is the
          one-hot-at-s weight matrix; wv is split hi/lo (two passes) so its
          fp32r rounding cancels; probs transposes; final attn @ V in
          32-column strips per group (fp32)
  DVE/ACT: softmax max / exp(accum_out row sums) / reciprocal, final scale
"""
import math
import os
import sys

import numpy as np

for _p in ("/opt/trn_rl_repo", "/root/.axon_site/_ro/trn_rl_repo"):
    if os.path.isdir(_p):
        if _p not in sys.path:
            sys.path.insert(0, _p)
        break

B, Q, KV, QS, KS, H, VS = 4, 256, 1024, 128, 128, 128, 256
P = 128
N_CORES = 8
GROUP_ROWS = 32          # rows per (core, batch)
SUB = 8                  # rows per tanh batch

PROFILE = False          # set by test.py; enables NTFF tracing
LO_PASS = True           # wv hi/lo split second matvec pass (precision)
LAST_RESULTS = None
SIMULATE = False         # set by test.py; run CoreSim instead of hardware
LAST_EXEC_NS = None

_prog_cache = {}


def _build_program(cfg):
    """cfg: (ncfg, l0flags): per-group KV chunk counts (sorted desc) and
    per-group valid_len==0 flags. Returns nc."""
    Ws, l0flags, _lo = cfg
    ncfg = [(w + P - 1) // P for w in Ws]
    import contextlib

    import concourse.bacc as bacc
    import concourse.mybir as mybir
    import concourse.tile as tile
    from concourse.tile_rust import add_dep_helper

    f32 = mybir.dt.float32
    W = list(Ws)                        # per-group computed KV width
    Wmax = W[0]
    nc = bacc.Bacc("TRN2", target_bir_lowering=False, debug=False,
                   enable_asserts=True, num_devices=N_CORES)

    blob_d = nc.dram_tensor("blob", [P, 3 * P], f32,
                            kind="ExternalInput").ap()
    ident_d = nc.dram_tensor("ident", [P, P], f32,
                             kind="ExternalInput").ap()
    wvdb_d = nc.dram_tensor("wvdb", [P, 2 * (2 * P - 1)], mybir.dt.float32r,
                            kind="ExternalInput").ap()
    kT_d = nc.dram_tensor("kT", [P, B * KV], f32, kind="ExternalInput").ap()
    V_d = nc.dram_tensor("V", [B, KV, VS], f32, kind="ExternalInput").ap()
    ind_d = nc.dram_tensor("ind", [B, P], mybir.dt.float32r,
                           kind="ExternalInput").ap()
    wvd0_d = nc.dram_tensor("wvd0", [P, 2 * P - 1], mybir.dt.float32r,
                            kind="ExternalInput").ap()
    mask_d = nc.dram_tensor("mask", [B, Wmax], mybir.dt.float32r,
                            kind="ExternalInput").ap()
    out_d = nc.dram_tensor("out", [P, VS], f32, kind="ExternalOutput").ap()

    with tile.TileContext(nc) as tc, contextlib.ExitStack() as ctx:
        const = ctx.enter_context(tc.tile_pool(name="const", bufs=1))
        ktp = ctx.enter_context(tc.tile_pool(name="ktp", bufs=2))
        vbytes = sum((w + P - 1) // P for w in W)          # V tiles, KB/part
        feats_kb = SUB * Wmax * 4 / 1024
        feats_bufs = max(2, min(5, int((192 - 50 - vbytes - 16) // feats_kb)))
        feats_pool = ctx.enter_context(
            tc.tile_pool(name="featsp", bufs=feats_bufs))
        small = ctx.enter_context(tc.tile_pool(name="small", bufs=1))
        psum = ctx.enter_context(tc.tile_pool(name="psum", bufs=1, space="PSUM"))
        spsum = ctx.enter_context(tc.tile_pool(name="spsum", bufs=3, space="PSUM"))
        psum2 = ctx.enter_context(tc.tile_pool(name="psum2", bufs=1, space="PSUM"))

        # ---- constant loads: one blob DMA on the critical Sync queue;
        # small leftovers go through the idle GpSimd issue queue ----
        f32r = mybir.dt.float32r
        blob = const.tile([P, 3 * P], f32)
        nc.sync.dma_start(blob[:], blob_d[:])
        qt_sb = blob[:, 0:P]
        wq_sb = blob[:, P:2 * P]
        wk_sb = blob[:, 2 * P:3 * P]
        ident_t = const.tile([P, P], f32)
        nc.gpsimd.dma_start(ident_t[:], ident_d[:])
        ident_sb = ident_t[:]
        wvdb = const.tile([P, 2 * (2 * P - 1)], f32r)
        nc.gpsimd.dma_start(wvdb[:], wvdb_d[:])
        wvd_hi = wvdb[:, 0:2 * P - 1]
        wvd_lo = wvdb[:, 2 * P - 1:]
        if any(l0flags):
            wvd0_t = const.tile([P, 2 * P - 1], f32r)
            nc.gpsimd.dma_start(wvd0_t[:], wvd0_d[:])
            wvd0 = wvd0_t[:]
        ind_sb = const.tile([B, P], f32r)
        nc.gpsimd.dma_start(ind_sb[:], ind_d[:])
        mask_sb = const.tile([B, Wmax], f32r)
        nc.gpsimd.dma_start(mask_sb[:], mask_d[:])

        # ---- single shared scores PSUM tile; per-group projections are
        # emitted just before each group's batches ----
        scores_ps = psum.tile([P, Wmax], f32, name="scores_ps")
        vts = {}

        # ---- q projection (reuses the pt/qp psum bank) ----
        qp_ps = psum2.tile([P, P], f32, tag="ptqp", bufs=3, name="qp_ps")
        nc.tensor.matmul(qp_ps[:], wq_sb[:], qt_sb[:], start=True, stop=True)
        qp_sb = const.tile([P, P], f32)
        nc.vector.tensor_copy(qp_sb[:], qp_ps[:])

        kp_sb = const.tile([P, B * KV], f32)
        g_order = sorted(range(B), key=lambda g: W[g])

        def proj_group(g):
            cp = None
            for j in range(0, W[g], 512):
                n = min(512, W[g] - j)
                kt_t = ktp.tile([P, 512], f32, tag="kt", name=f"kt_{g}_{j}",
                                bufs=4)
                nc.sync.dma_start(kt_t[:, :n], kT_d[:, g * KV + j: g * KV + j + n])
                kp_ps = psum2.tile([P, 512], f32, tag="proj", bufs=2,
                                   name=f"kp_ps_{g}_{j}")
                nc.tensor.matmul(kp_ps[:, :n], wk_sb[:], kt_t[:, :n],
                                 start=True, stop=True)
                cp = nc.vector.tensor_copy(
                    kp_sb[:, g * KV + j: g * KV + j + n], kp_ps[:, :n])
            return cp

        # masks first (start=True initializes each bank): one K=B matmul
        # per 512-col chunk writes every group's mask row onto its band
        for c0 in range(0, Wmax, 512):
            c1 = min(c0 + 512, Wmax)
            nc.tensor.matmul(scores_ps[:, c0:c1], ind_sb[:], mask_sb[:, c0:c1],
                             start=True, stop=False, skip_group_check=True)

        for gi, g in enumerate(g_order):
            wg = W[g]
            last_kp_copy = proj_group(g)
            wsrcs = [wvd0] if l0flags[g] else ([wvd_hi, wvd_lo]
                     if _lo else [wvd_hi])
            for sb in range(GROUP_ROWS // SUB):
                feats = feats_pool.tile([P, SUB * wg], f32r, tag="feats",
                                        name=f"feats_{g}_{sb}")
                for j in range(SUB):
                    s = GROUP_ROWS * g + SUB * sb + j
                    nc.vector.tensor_scalar_add(
                        feats[:, j * wg:(j + 1) * wg],
                        kp_sb[:, g * KV: g * KV + wg],
                        qp_sb[:, s: s + 1])
                nc.scalar.activation(feats[:], feats[:],
                                     mybir.ActivationFunctionType.Tanh)
                for j in range(SUB):
                    s = GROUP_ROWS * g + SUB * sb + j
                    last = (gi == B - 1 and sb == GROUP_ROWS // SUB - 1
                            and j == SUB - 1)
                    for wsrc in wsrcs:
                        for c0 in range(0, wg, 512):
                            c1 = min(c0 + 512, wg)
                            nc.tensor.matmul(
                                scores_ps[:, c0:c1],
                                wsrc[:, P - 1 - s: 2 * P - 1 - s],
                                feats[:, j * wg + c0: j * wg + c1],
                                start=False,
                                stop=(last and wsrc is wsrcs[-1]
                                      and c0 + 512 >= wg),
                                skip_group_check=True)

        # ---- V tiles: ordered after all kp so kT wins head HBM bandwidth --
        for g in range(B):
            for c in range((W[g] + P - 1) // P):
                vts[(g, c)] = const.tile([P, VS], f32, name=f"v_{g}_{c}")
                vdma = nc.sync.dma_start(vts[(g, c)][:],
                                         V_d[g, c * P:(c + 1) * P, :])
                add_dep_helper(vdma.ins, last_kp_copy.ins,
                               reason="V after kp: kT wins head HBM bw")

        # ---- softmax ----
        nrowmax = small.tile([P, 1], f32)
        nc.vector.reduce_max(nrowmax[:], scores_ps[:, :Wmax],
                             axis=mybir.AxisListType.X, negate=True)
        probs = small.tile([P, Wmax], f32)
        n_ec = (Wmax + 255) // 256
        psums = small.tile([P, n_ec], f32)
        for e in range(n_ec):
            e0, e1 = e * 256, min((e + 1) * 256, Wmax)
            nc.scalar.activation(probs[:, e0:e1], scores_ps[:, e0:e1],
                                 mybir.ActivationFunctionType.Exp,
                                 bias=nrowmax[:, 0:1], scale=1.0,
                                 accum_out=psums[:, e:e + 1])
        rowsum = small.tile([P, 1], f32)
        nc.vector.reduce_sum(rowsum[:], psums[:], axis=mybir.AxisListType.X)
        rinv = small.tile([P, 1], f32)
        nc.vector.reciprocal(rinv[:], rowsum[:])

        out_ps = psum.tile([P, VS], f32, name="out_ps")
        for c in range(ncfg[0]):
            cw = min(P, Wmax - c * P)
            pt_ps = psum2.tile([P, P], f32, tag="ptqp", bufs=3,
                               name=f"pt_ps{c}")
            nc.tensor.transpose(pt_ps[:cw, :], probs[:, c * P:c * P + cw],
                                ident_sb[:])
            pt_sb = small.tile([P, P], f32, name=f"pt_sb{c}")
            nc.vector.tensor_copy(pt_sb[:cw, :], pt_ps[:cw, :])
            for g in range(B):
                if c * P < W[g]:
                    gw = min(P, W[g] - c * P)
                    nc.tensor.matmul(
                        out_ps[GROUP_ROWS * g: GROUP_ROWS * (g + 1), :],
                        pt_sb[:gw, GROUP_ROWS * g: GROUP_ROWS * (g + 1)],
                        vts[(g, c)][:gw, :],
                        start=(c == 0), stop=(c == (W[g] + P - 1) // P - 1),
                        tile_position=(0, GROUP_ROWS * g),
                        skip_group_check=True)

        out_sb = small.tile([P, VS], f32)
        nc.vector.tensor_scalar_mul(out_sb[:], out_ps[:], rinv[:, 0:1])
        nc.sync.dma_start(out_d[:], out_sb[:])

    nc.compile()
    return nc


def _get_program(ncfg):
    if ncfg not in _prog_cache:
        _prog_cache[ncfg] = _build_program(ncfg)
    return _prog_cache[ncfg]


def kernel(queries, keys, values, valid_lens, Wq, Wk, wv):
    global LAST_EXEC_NS
    queries = np.ascontiguousarray(np.asarray(queries), dtype=np.float32)
    keys = np.ascontiguousarray(np.asarray(keys), dtype=np.float32)
    values = np.ascontiguousarray(np.asarray(values), dtype=np.float32)
    Wq = np.ascontiguousarray(np.asarray(Wq), dtype=np.float32)
    Wk = np.ascontiguousarray(np.asarray(Wk), dtype=np.float32)
    wv = np.ascontiguousarray(np.asarray(wv), dtype=np.float32)
    vl = [int(x) for x in np.asarray(valid_lens)]

    def width(L):
        # fp32r matmul chunks must be >= 256 cols; widths are 32-multiples
        if L <= 0:
            return KV
        L = min(L, KV)
        if L <= 512:
            return max(256, 32 * math.ceil(L / 32))
        return 512 + max(256, 32 * math.ceil((L - 512) / 32))

    W_b = [width(L) for L in vl]
    order = sorted(range(B), key=lambda b: (-W_b[b], b))
    Ws = tuple(W_b[b] for b in order)
    ncfg = [(w + P - 1) // P for w in Ws]
    l0flags = tuple(vl[order[g]] == 0 for g in range(B))
    Wmax = Ws[0]

    nc = _get_program((Ws, l0flags, LO_PASS))

    kT = np.concatenate([keys[order[g]].T for g in range(B)], axis=1)
    kT = np.ascontiguousarray(kT)                        # [128, 4096]
    Vm = np.ascontiguousarray(np.stack([values[order[g]] for g in range(B)]))
    ind = np.zeros((B, P), np.float32)
    for g in range(B):
        ind[g, GROUP_ROWS * g: GROUP_ROWS * (g + 1)] = 1.0
    mask = np.full((B, Wmax), -1e6, np.float32)
    for g in range(B):
        L = vl[order[g]]
        if L > 0:
            mask[g, :min(L, Wmax)] = 0.0
        else:
            mask[g, :] = 0.0
    ident = np.eye(P, dtype=np.float32)

    import ml_dtypes
    bf16 = ml_dtypes.bfloat16
    wv_hi = wv.astype(bf16).astype(np.float32)
    DW = 2 * P - 1
    blob = np.zeros((P, 3 * P), np.float32)
    blob[:, P:2 * P] = Wq
    blob[:, 2 * P:3 * P] = Wk
    wvdb = np.zeros((P, 2 * DW), np.float32)
    wvdb[:, P - 1] = wv_hi if LO_PASS else wv
    wvdb[:, DW + P - 1] = wv - wv_hi
    wvd0 = np.zeros((P, DW), np.float32)
    shared = {"kT": kT, "V": Vm, "ind": ind, "mask": mask, "wvd0": wvd0,
              "wvdb": wvdb, "ident": ident}
    in_maps = []
    for c in range(N_CORES):
        qT = np.concatenate(
            [queries[order[g], c * GROUP_ROWS:(c + 1) * GROUP_ROWS, :].T
             for g in range(B)], axis=1)
        bl = blob.copy()
        bl[:, 0:P] = qT
        m = dict(shared)
        m["blob"] = bl
        in_maps.append(m)

    if SIMULATE:
        from concourse.bass_interp import CoreSim
        outs = []
        for c in range(N_CORES):
            sim = CoreSim(nc, trace=False)
            for name, v in in_maps[c].items():
                sim.tensor(name)[:] = v
            sim.simulate(check_with_hw=False)
            outs.append(sim.tensor("out").copy())
    else:
        from concourse import bass_utils
        kw = {}
        if PROFILE:
            kw = {"trace": True}
        res = bass_utils.run_bass_kernel_spmd(nc, in_maps, list(range(N_CORES)),
                                              **kw)
        if PROFILE:
            LAST_EXEC_NS = res.exec_time_ns
            global LAST_RESULTS
            LAST_RESULTS = res
        outs = [res.results[c]["out"] for c in range(N_CORES)]

    out = np.zeros((B, Q, VS), np.float32)
    for c in range(N_CORES):
        for g in range(B):
            out[order[g], c * GROUP_ROWS:(c + 1) * GROUP_ROWS, :] = \
                outs[c][GROUP_ROWS * g: GROUP_ROWS * (g + 1), :]
    return out



# revision 3
# speedup vs baseline: 1.1536x; 1.1536x over previous
"""Additive attention (B=4, Q=256, KV=1024, H=128, VS=256) on 8 Trainium2 cores.

Sharding: each core processes 32 query rows of every batch (4 groups of 32
row-slots).  Per batch, only a KV prefix of width ~valid_len (padded to even)
is computed; masked columns beyond it contribute exactly 0 to the softmax, so
skipping them is exact.  No collectives.  The program is specialized per
valid_lens configuration at call time and cached.

Per-core dataflow (ACT tanh is the hard floor: 1 elem/cycle/lane,
dtype-independent, ScalarE-only):
  PE  : k/q projections in fp32 (accuracy-critical, pre-tanh)
  DVE : feats[h, kv] = fp16(kp16[h, kv] + qp[h, s])  (tensor_scalar add,
        fp16 4x mode; kp stored fp16, qp scalar fp32)
  ACT : tanh in place over 8-row batches (the throughput floor, ~50us)
  PE  : single-pass fp16 one-hot matmuls accumulate score rows into a
        per-group PSUM tile (wv window with the weight at column 127-s);
        each group's mask row is written first via one K=1 matmul
  DVE/ACT/DMA: per-group masked softmax (reduce_max / exp-with-accum-out row
        sums -> fp16 probs), probs transposed by the DMA xbar (idle engine),
        attn @ V in fp16 32-column bands; group i's softmax+attnV hide under
        group i+1's tanh stream.  One reciprocal + scale at the very end.
Queue discipline: all kT DMAs + projections are emitted up front (PE/DVE
queues never stall behind the ACT-paced score matmuls); V-tile DMAs trail kT
on the sync queue; softmax of group i is emitted after the tanh stream of
group i+1 so no engine queue blocks on a cross-engine round trip.
"""
import math
import os
import sys

import numpy as np

for _p in ("/opt/trn_rl_repo", "/root/.axon_site/_ro/trn_rl_repo"):
    if os.path.isdir(_p):
        if _p not in sys.path:
            sys.path.insert(0, _p)
        break

B, Q, KV, QS, KS, H, VS = 4, 256, 1024, 128, 128, 128, 256
P = 128
N_CORES = 8
GROUP_ROWS = 32          # rows per (core, batch)
SUB = 8                  # rows per tanh batch
MASK_VAL = -30000.0      # large-negative that still fits fp16

PROFILE = False          # set by test.py; enables NTFF tracing
LO_PASS = True           # kept for test.py compat (unused in v2)
LAST_RESULTS = None
SIMULATE = False         # set by test.py; run CoreSim instead of hardware
LAST_EXEC_NS = None

_prog_cache = {}


def _build_program(cfg):
    """cfg: (Ws, l0flags): per-group computed KV widths in processing order
    and per-group valid_len==0 flags.  Returns nc."""
    Ws, l0flags = cfg
    import contextlib

    import concourse.bacc as bacc
    import concourse.mybir as mybir
    import concourse.tile as tile
    from concourse.tile_rust import add_dep_helper

    f32 = mybir.dt.float32
    f16 = mybir.dt.float16
    W = list(Ws)
    Wmax = max(W)
    SW = sum(W)
    offs = [sum(W[:i]) for i in range(B)]          # kp_sb column offsets
    nstrips = [(w + P - 1) // P for w in W]
    nc = bacc.Bacc("TRN2", target_bir_lowering=False, debug=False,
                   enable_asserts=True, num_devices=N_CORES)

    blob_d = nc.dram_tensor("blob", [P, 3 * P], f32, kind="ExternalInput").ap()
    kT_d = nc.dram_tensor("kT", [P, SW], f32, kind="ExternalInput").ap()
    V_d = nc.dram_tensor("V", [B, KV, VS], f16, kind="ExternalInput").ap()
    wvd_d = nc.dram_tensor("wvd", [P, 2 * P - 1], f16, kind="ExternalInput").ap()
    ind_d = nc.dram_tensor("ind", [1, B * P], f16, kind="ExternalInput").ap()
    mask_d = nc.dram_tensor("mask", [1, B * Wmax], f16, kind="ExternalInput").ap()
    out_d = nc.dram_tensor("out", [P, VS], f32, kind="ExternalOutput").ap()

    with tile.TileContext(nc) as tc, contextlib.ExitStack() as ctx:
        const = ctx.enter_context(tc.tile_pool(name="const", bufs=1))
        ktp = ctx.enter_context(tc.tile_pool(name="ktp", bufs=4))
        featsp = ctx.enter_context(tc.tile_pool(name="featsp", bufs=5))
        probsp = ctx.enter_context(tc.tile_pool(name="probsp", bufs=2))
        small = ctx.enter_context(tc.tile_pool(name="small", bufs=3))
        scp = ctx.enter_context(tc.tile_pool(name="scp", bufs=2, space="PSUM"))
        pmix = ctx.enter_context(tc.tile_pool(name="pmix", bufs=3, space="PSUM"))
        outp = ctx.enter_context(tc.tile_pool(name="outp", bufs=1, space="PSUM"))

        # ---- ACT table warm-up: load the exp/tanh spline set while the
        # first DMAs are still in flight ----
        warm = const.tile([1, 2], f16)
        nc.gpsimd.memset(warm[:], 0.0)
        nc.scalar.activation(warm[:], warm[:],
                             mybir.ActivationFunctionType.Tanh)

        # ---- constant loads: blob + kT + V on the Sync queue (in priority
        # order); small leftovers on the idle GpSimd issue queue ----
        blob = const.tile([P, 3 * P], f32)
        nc.sync.dma_start(blob[:], blob_d[:])
        qt_sb = blob[:, 0:P]
        wq_sb = blob[:, P:2 * P]
        wk_sb = blob[:, 2 * P:3 * P]
        wvd_t = const.tile([P, 2 * P - 1], f16)
        nc.gpsimd.dma_start(wvd_t[:], wvd_d[:])
        ind_sb = const.tile([1, B * P], f16)
        nc.gpsimd.dma_start(ind_sb[:], ind_d[:])
        mask_sb = const.tile([1, B * Wmax], f16)
        nc.gpsimd.dma_start(mask_sb[:], mask_d[:])

        # ---- q projection ----
        qp_ps = pmix.tile([P, P], f32, tag="mix", name="qp_ps")
        nc.tensor.matmul(qp_ps[:], wq_sb[:], qt_sb[:], start=True, stop=True)
        qp_sb = const.tile([P, P], f32)
        nc.vector.tensor_copy(qp_sb[:], qp_ps[:])

        kp_sb = const.tile([P, SW], f16)
        out_ps = outp.tile([P, VS], f32, name="out_ps")
        nrowmax = small.tile([P, 1], f32, bufs=1, tag="nrm")
        rowsum = small.tile([P, 1], f32, bufs=1, tag="rsum")
        rinv = small.tile([P, 1], f32, bufs=1, tag="rinv")
        vts = {}
        scores = [None] * B
        probs = [None] * B
        last_kp_copy = [None]

        def emit_load(i):
            """kT DMA + fp32 projection + fp16 kp copy for group i."""
            w = W[i]
            for c0 in range(0, w, 512):
                n = min(512, w - c0)
                kt_t = ktp.tile([P, 512], f32, tag="kt", name=f"kt_{i}_{c0}")
                nc.sync.dma_start(kt_t[:, :n], kT_d[:, offs[i] + c0:
                                                   offs[i] + c0 + n])
                kp_ps = pmix.tile([P, 512], f32, tag="mix",
                                  name=f"kp_ps_{i}_{c0}")
                nc.tensor.matmul(kp_ps[:, :n], wk_sb[:], kt_t[:, :n],
                                 start=True, stop=True)
                last_kp_copy[0] = nc.vector.tensor_copy(
                    kp_sb[:, offs[i] + c0: offs[i] + c0 + n], kp_ps[:, :n])

        def emit_mask(i):
            w = W[i]
            sc = scp.tile([P, w], f32, tag="sc", name=f"scores_{i}")
            scores[i] = sc
            for c0 in range(0, w, 512):
                c1 = min(c0 + 512, w)
                nc.tensor.matmul(
                    sc[:, c0:c1],
                    ind_sb[0:1, i * P:(i + 1) * P],
                    mask_sb[0:1, i * Wmax + c0: i * Wmax + c1],
                    start=True, stop=l0flags[i] and c1 == w,
                    skip_group_check=True)

        def emit_scores(i, sbs):
            """adds + tanh + one-hot score matmuls for group i, sub-batches
            sbs."""
            w = W[i]
            sc = scores[i]
            for sb in sbs:
                feats = featsp.tile([P, SUB * w], f16, tag="feats",
                                    name=f"feats_{i}_{sb}")
                for j in range(SUB):
                    s = GROUP_ROWS * i + SUB * sb + j
                    nc.vector.tensor_scalar_add(
                        feats[:, j * w:(j + 1) * w],
                        kp_sb[:, offs[i]: offs[i] + w],
                        qp_sb[:, s: s + 1])
                nc.scalar.activation(feats[:], feats[:],
                                     mybir.ActivationFunctionType.Tanh)
                for j in range(SUB):
                    s = GROUP_ROWS * i + SUB * sb + j
                    last_row = sb == GROUP_ROWS // SUB - 1 and j == SUB - 1
                    for c0 in range(0, w, 512):
                        c1 = min(c0 + 512, w)
                        nc.tensor.matmul(
                            sc[:, c0:c1],
                            wvd_t[:, P - 1 - s: 2 * P - 1 - s],
                            feats[:, j * w + c0: j * w + c1],
                            start=False,
                            stop=last_row and c1 == w,
                            skip_group_check=True)

        def emit_vdma(i):
            for c in range(nstrips[i]):
                cw = min(P, W[i] - c * P)
                vts[(i, c)] = const.tile([P, VS], f16, name=f"v_{i}_{c}")
                vdma = nc.sync.dma_start(vts[(i, c)][:cw, :],
                                         V_d[i, c * P: c * P + cw, :])
                if last_kp_copy[0] is not None:
                    add_dep_helper(vdma.ins, last_kp_copy[0].ins,
                                   reason="V after kp: kT wins head HBM bw")

        def emit_softmax_attnv(i):
            w = W[i]
            wpad = nstrips[i] * P
            sc = scores[i]
            band = slice(GROUP_ROWS * i, GROUP_ROWS * (i + 1))
            nc.vector.reduce_max(nrowmax[band, :], sc[band, :],
                                 axis=mybir.AxisListType.X, negate=True)
            pr = probsp.tile([P, wpad], f16, tag="probs", name=f"probs_{i}")
            probs[i] = pr
            if wpad > w:
                nc.vector.memset(pr[band, w:wpad], 0.0)
            nc.scalar.activation(pr[band, :w], sc[band, :],
                                 mybir.ActivationFunctionType.Exp,
                                 bias=nrowmax[band, 0:1], scale=1.0,
                                 accum_out=rowsum[band, 0:1])
            for c in range(nstrips[i]):
                cw = min(P, w - c * P)
                pt_sb = small.tile([P, GROUP_ROWS], f16, tag="pt",
                                   name=f"pt_sb_{i}_{c}")
                nc.sync.dma_start_transpose(pt_sb[:],
                                            pr[band, c * P:(c + 1) * P])
                nc.tensor.matmul(
                    out_ps[band, :],
                    pt_sb[:cw, :],
                    vts[(i, c)][:cw, :],
                    start=(c == 0), stop=(c == nstrips[i] - 1),
                    tile_position=(0, GROUP_ROWS * i),
                    skip_group_check=True)

        # ---- emission: kT loads first (groups 2,3 after the first tanh
        # sub-batch so group 0's adds start early), then the ACT-paced main
        # loop with group i's softmax hidden under group i+1's tanh ----
        emit_load(0)
        emit_load(1)
        emit_mask(0)
        if not l0flags[0]:
            emit_scores(0, [0])
        emit_load(2)
        emit_load(3)
        if not l0flags[0]:
            emit_scores(0, [1, 2, 3])
        emit_vdma(0)
        for i in range(1, B):
            emit_mask(i)
            if not l0flags[i]:
                emit_scores(i, range(GROUP_ROWS // SUB))
            emit_vdma(i)
            emit_softmax_attnv(i - 1)
        emit_softmax_attnv(B - 1)

        nc.vector.reciprocal(rinv[:], rowsum[:])
        out_sb = const.tile([P, VS], f32)
        nc.vector.tensor_scalar_mul(out_sb[:], out_ps[:], rinv[:, 0:1])
        nc.sync.dma_start(out_d[:], out_sb[:])

    nc.compile()
    return nc


def _get_program(cfg):
    if cfg not in _prog_cache:
        _prog_cache[cfg] = _build_program(cfg)
    return _prog_cache[cfg]


def _width(L):
    # even-padded computed width; valid_len==0 means "uniform over all KV"
    if L <= 0:
        return KV
    L = min(L, KV)
    return min(KV, max(2, 2 * math.ceil(L / 2)))


def kernel(queries, keys, values, valid_lens, Wq, Wk, wv):
    global LAST_EXEC_NS
    queries = np.ascontiguousarray(np.asarray(queries), dtype=np.float32)
    keys = np.ascontiguousarray(np.asarray(keys), dtype=np.float32)
    values = np.ascontiguousarray(np.asarray(values), dtype=np.float32)
    Wq = np.ascontiguousarray(np.asarray(Wq), dtype=np.float32)
    Wk = np.ascontiguousarray(np.asarray(Wk), dtype=np.float32)
    wv = np.ascontiguousarray(np.asarray(wv), dtype=np.float32)
    vl = [int(x) for x in np.asarray(valid_lens)]

    W_b = [_width(L) for L in vl]
    desc = sorted(range(B), key=lambda b: (-W_b[b], b))
    perm = [2, 0, 1, 3]          # fast pipeline start, smallest group last
    gorder = [desc[p] for p in perm]
    Ws = tuple(W_b[b] for b in gorder)
    l0flags = tuple(vl[b] == 0 for b in gorder)
    Wmax = max(Ws)

    nc = _get_program((Ws, l0flags))

    kT = np.concatenate(
        [keys[gorder[i]][:Ws[i]].T for i in range(B)], axis=1)
    kT = np.ascontiguousarray(kT)                        # [128, SW]
    Vm = np.ascontiguousarray(
        np.stack([values[gorder[i]] for i in range(B)]).astype(np.float16))
    ind = np.zeros((1, B * P), np.float16)
    for i in range(B):
        ind[0, i * P + GROUP_ROWS * i: i * P + GROUP_ROWS * (i + 1)] = 1.0
    mask = np.zeros((1, B * Wmax), np.float16)
    for i in range(B):
        L = vl[gorder[i]]
        if L > 0:
            mask[0, i * Wmax + min(L, Ws[i]): i * Wmax + Ws[i]] = MASK_VAL
    wvd = np.zeros((P, 2 * P - 1), np.float16)
    wvd[:, P - 1] = wv.astype(np.float16)

    blob = np.zeros((P, 3 * P), np.float32)
    blob[:, P:2 * P] = Wq
    blob[:, 2 * P:3 * P] = Wk
    shared = {"kT": kT, "V": Vm, "ind": ind, "mask": mask, "wvd": wvd}
    in_maps = []
    for c in range(N_CORES):
        qT = np.concatenate(
            [queries[gorder[i], c * GROUP_ROWS:(c + 1) * GROUP_ROWS, :].T
             for i in range(B)], axis=1)
        bl = blob.copy()
        bl[:, 0:P] = qT
        m = dict(shared)
        m["blob"] = bl
        in_maps.append(m)

    if SIMULATE:
        from concourse.bass_interp import CoreSim
        outs = []
        for c in range(N_CORES):
            sim = CoreSim(nc, trace=False)
            for name, v in in_maps[c].items():
                sim.tensor(name)[:] = v
            sim.simulate(check_with_hw=False)
            outs.append(sim.tensor("out").copy())
    else:
        from concourse import bass_utils
        kw = {}
        if PROFILE:
            kw = {"trace": True}
        res = bass_utils.run_bass_kernel_spmd(nc, in_maps, list(range(N_CORES)),
                                              **kw)
        if PROFILE:
            LAST_EXEC_NS = res.exec_time_ns
            global LAST_RESULTS
            LAST_RESULTS = res
        outs = [res.results[c]["out"] for c in range(N_CORES)]

    out = np.zeros((B, Q, VS), np.float32)
    for c in range(N_CORES):
        for i in range(B):
            out[gorder[i], c * GROUP_ROWS:(c + 1) * GROUP_ROWS, :] = \
                outs[c][GROUP_ROWS * i: GROUP_ROWS * (i + 1), :]
    return out


# revision 11
# speedup vs baseline: 1.2764x; 1.1065x over previous
"""Additive attention (B=4, Q=256, KV=1024, H=128, VS=256) on 8 Trainium2 cores.

Sharding: each core processes 32 query rows of every batch (4 groups of 32
row-slots).  Per batch, only a KV prefix of width ~valid_len (padded to even)
is computed; masked columns beyond it contribute exactly 0 to the softmax, so
skipping them is exact.  No collectives.  The program is specialized per
valid_lens configuration at call time and cached.

Per-core dataflow (ACT tanh is the hard floor: 1 elem/cycle/lane,
dtype-independent, ScalarE-only):
  PE  : k/q projections in fp32 (accuracy-critical, pre-tanh)
  DVE : feats[h, kv] = fp16(kp16[h, kv] + qp[h, s])  (tensor_scalar add,
        fp16 4x mode; kp stored fp16, qp scalar fp32)
  ACT : tanh in place over 8-row batches (the throughput floor, ~50us)
  PE  : single-pass fp16 one-hot matmuls accumulate score rows into a
        per-group PSUM tile (wv window with the weight at column 127-s);
        each group's mask row is written first via one K=1 matmul
  DVE/ACT/PE: per-group masked softmax (reduce_max / exp-with-accum-out row
        sums -> fp32 probs), PE band transposes into one PSUM strip tile,
        one fp16 cast copy, attn @ V in fp16 32-column bands; group i's
        softmax+attnV hide under group i+1's tanh stream.  One reciprocal +
        scale at the very end.
Queue discipline: Wk arrives in its own first DMA so the k0 projection can
start immediately; kT loads are interleaved with group 0's adds; softmax of
group i is emitted after sub-batch 0 of group i+1's tanh so no engine queue
blocks on a cross-engine round trip; V-tile DMAs trail kT on the sync queue.
"""
import math
import os
import sys

import numpy as np

for _p in ("/opt/trn_rl_repo", "/root/.axon_site/_ro/trn_rl_repo"):
    if os.path.isdir(_p):
        if _p not in sys.path:
            sys.path.insert(0, _p)
        break

B, Q, KV, QS, KS, H, VS = 4, 256, 1024, 128, 128, 128, 256
P = 128
N_CORES = 8
GROUP_ROWS = 32          # rows per (core, batch)
SUB = 8                  # rows per tanh batch
MASK_VAL = -30000.0      # large-negative that still fits fp16

PROFILE = False          # set by test.py; enables NTFF tracing
LO_PASS = True           # kept for test.py compat (unused in v2)
LAST_RESULTS = None
SIMULATE = False         # set by test.py; run CoreSim instead of hardware
LAST_EXEC_NS = None

_prog_cache = {}


def _build_program(cfg):
    """cfg: (Ws, l0flags): per-group computed KV widths in processing order
    and per-group valid_len==0 flags.  Returns nc."""
    Ws, l0flags = cfg
    import contextlib

    import concourse.bacc as bacc
    import concourse.mybir as mybir
    import concourse.tile as tile
    from concourse.tile_rust import add_dep_helper

    f32 = mybir.dt.float32
    f16 = mybir.dt.float16
    W = list(Ws)
    Wmax = max(W)
    SW = sum(W)
    offs = [sum(W[:i]) for i in range(B)]          # kp_sb column offsets
    nstrips = [(w + P - 1) // P for w in W]
    NSB = GROUP_ROWS // SUB
    nc = bacc.Bacc("TRN2", target_bir_lowering=False, debug=False,
                   enable_asserts=True, num_devices=N_CORES)

    wk_d = nc.dram_tensor("wk", [P, P], f32, kind="ExternalInput").ap()
    blobq_d = nc.dram_tensor("blobq", [P, 2 * P], f32,
                             kind="ExternalInput").ap()
    kT_d = nc.dram_tensor("kT", [P, SW], f32, kind="ExternalInput").ap()
    V_d = nc.dram_tensor("V", [B, KV, VS], f16, kind="ExternalInput").ap()
    wvd_d = nc.dram_tensor("wvd", [P, 2 * P - 1], f16, kind="ExternalInput").ap()
    ind_d = nc.dram_tensor("ind", [2, B * P], f16, kind="ExternalInput").ap()
    mask_d = nc.dram_tensor("mask", [2, B * Wmax], f16, kind="ExternalInput").ap()
    bident_d = nc.dram_tensor("bident", [P, GROUP_ROWS], f16,
                              kind="ExternalInput").ap()
    out_d = nc.dram_tensor("out", [P, VS], f32, kind="ExternalOutput").ap()

    with tile.TileContext(nc) as tc, contextlib.ExitStack() as ctx:
        const = ctx.enter_context(tc.tile_pool(name="const", bufs=1))
        ktp = ctx.enter_context(tc.tile_pool(name="ktp", bufs=4))
        featsp = ctx.enter_context(tc.tile_pool(name="featsp", bufs=5))
        probsp = ctx.enter_context(tc.tile_pool(name="probsp", bufs=2))
        small = ctx.enter_context(tc.tile_pool(name="small", bufs=3))
        scp = ctx.enter_context(tc.tile_pool(name="scp", bufs=2, space="PSUM"))
        pmix = ctx.enter_context(tc.tile_pool(name="pmix", bufs=3, space="PSUM"))
        outp = ctx.enter_context(tc.tile_pool(name="outp", bufs=1, space="PSUM"))

        # ---- ACT table warm-up: load the exp/tanh spline set while the
        # first DMAs are still in flight ----
        warm = const.tile([1, 2], f16)
        nc.gpsimd.memset(warm[:], 0.0)
        nc.scalar.activation(warm[:], warm[:],
                             mybir.ActivationFunctionType.Tanh)

        # ---- constant loads: wk first (k0 projection gates the pipeline),
        # then qT|Wq, then kT chunks, then V tiles on the Sync queue; small
        # leftovers on the idle GpSimd issue queue ----
        wk_sb_t = const.tile([P, P], f32)
        nc.sync.dma_start(wk_sb_t[:], wk_d[:])
        wk_sb = wk_sb_t[:]
        blobq = const.tile([P, 2 * P], f32)
        nc.sync.dma_start(blobq[:], blobq_d[:])
        qt_sb = blobq[:, 0:P]
        wq_sb = blobq[:, P:2 * P]
        wvd_t = const.tile([P, 2 * P - 1], f16)
        nc.gpsimd.dma_start(wvd_t[:], wvd_d[:])
        ind_sb = const.tile([2, B * P], f16)
        nc.gpsimd.dma_start(ind_sb[:], ind_d[:])
        mask_sb = const.tile([2, B * Wmax], f16)
        nc.gpsimd.dma_start(mask_sb[:], mask_d[:])
        bident = const.tile([P, GROUP_ROWS], f16)
        nc.gpsimd.dma_start(bident[:], bident_d[:])

        kp_sb = const.tile([P, SW], f16)
        out_ps = outp.tile([P, VS], f32, name="out_ps")
        rinv = small.tile([P, 1], f32, bufs=1, tag="rinv")
        vts = {}
        scores = [None] * B
        last_kp_copy = [None]

        def emit_load(i):
            """kT DMA + fp32 projection + fp16 kp copy for group i."""
            w = W[i]
            for c0 in range(0, w, 512):
                n = min(512, w - c0)
                kt_t = ktp.tile([P, 512], f32, tag="kt", name=f"kt_{i}_{c0}")
                nc.sync.dma_start(kt_t[:, :n], kT_d[:, offs[i] + c0:
                                                   offs[i] + c0 + n])
                kp_ps = pmix.tile([P, 512], f32, tag="mix",
                                  name=f"kp_ps_{i}_{c0}")
                nc.tensor.matmul(kp_ps[:, :n], wk_sb, kt_t[:, :n],
                                 start=True, stop=True)
                last_kp_copy[0] = nc.vector.tensor_copy(
                    kp_sb[:, offs[i] + c0: offs[i] + c0 + n], kp_ps[:, :n])

        def emit_mask(i):
            # K=2 rank-2 init: rows in the band get the valid-len mask, rows
            # outside it get MASK_VAL so they exp to exactly 0 later (the
            # probs->pt reduction matmul sums over all four bands).
            w = W[i]
            sc = scp.tile([P, w], f32, tag="sc", name=f"scores_{i}")
            scores[i] = sc
            for c0 in range(0, w, 512):
                c1 = min(c0 + 512, w)
                nc.tensor.matmul(
                    sc[:, c0:c1],
                    ind_sb[0:2, i * P:(i + 1) * P],
                    mask_sb[0:2, i * Wmax + c0: i * Wmax + c1],
                    start=True, stop=l0flags[i] and c1 == w,
                    skip_group_check=True)

        def emit_scores(i, sbs):
            """adds + tanh + one-hot score matmuls for group i, sub-batches
            sbs."""
            w = W[i]
            sc = scores[i]
            for sb in sbs:
                feats = featsp.tile([P, SUB * w], f16, tag="feats",
                                    name=f"feats_{i}_{sb}")
                for j in range(SUB):
                    s = GROUP_ROWS * i + SUB * sb + j
                    nc.vector.tensor_scalar_add(
                        feats[:, j * w:(j + 1) * w],
                        kp_sb[:, offs[i]: offs[i] + w],
                        qp_sb[:, s: s + 1])
                nc.scalar.activation(feats[:], feats[:],
                                     mybir.ActivationFunctionType.Tanh)
                for j in range(SUB):
                    s = GROUP_ROWS * i + SUB * sb + j
                    last_row = sb == NSB - 1 and j == SUB - 1
                    for c0 in range(0, w, 512):
                        c1 = min(c0 + 512, w)
                        nc.tensor.matmul(
                            sc[:, c0:c1],
                            wvd_t[:, P - 1 - s: 2 * P - 1 - s],
                            feats[:, j * w + c0: j * w + c1],
                            start=False,
                            stop=last_row and c1 == w,
                            skip_group_check=True)

        def emit_vdma(i):
            for c in range(nstrips[i]):
                cw = min(P, W[i] - c * P)
                vts[(i, c)] = const.tile([P, VS], f16, name=f"v_{i}_{c}")
                vdma = nc.sync.dma_start(vts[(i, c)][:cw, :],
                                         V_d[i, c * P: c * P + cw, :])
                if last_kp_copy[0] is not None:
                    add_dep_helper(vdma.ins, last_kp_copy[0].ins,
                                   reason="V after kp: kT wins head HBM bw")

        def emit_softmax_attnv(i):
            w = W[i]
            n = nstrips[i]
            sc = scores[i]
            band = slice(GROUP_ROWS * i, GROUP_ROWS * (i + 1))
            # per-group -max; 0 outside the band so those rows (scores
            # MASK_VAL) exp to exactly 0
            nrm = small.tile([P, 1], f32, bufs=2, tag="nrm",
                             name=f"nrm_{i}")
            nc.vector.memset(nrm[:], 0.0)
            nc.vector.reduce_max(nrm[band, :], sc[band, :],
                                 axis=mybir.AxisListType.X, negate=True)
            wpad = n * P
            probs = probsp.tile([P, wpad], f16, tag="probs",
                                name=f"probs_{i}")
            rs = small.tile([P, 1], f32, bufs=4, tag="rs", name=f"rs_{i}")
            if wpad > w:
                nc.vector.memset(probs[:, w:], 0.0)
            nc.scalar.activation(probs[:, :w], sc[:],
                                 mybir.ActivationFunctionType.Exp,
                                 bias=nrm[:, 0:1], scale=1.0,
                                 accum_out=rs[:, 0:1])
            nc.vector.reciprocal(rinv[band, :], rs[band, :])
            # "transpose" probs via probs.T @ band-identity: full-height
            # stationary (base partition 0 -- quadrant-3-safe); non-band
            # rows are exactly 0 so the cross-band sum picks out the band
            pt_ps = pmix.tile([P, GROUP_ROWS * n], f32, tag="mix",
                              name=f"pt_ps_{i}")
            for c in range(n):
                nc.tensor.matmul(pt_ps[:, GROUP_ROWS * c:
                                       GROUP_ROWS * (c + 1)],
                                 probs[:, c * P:(c + 1) * P],
                                 bident[:, :],
                                 start=True, stop=True,
                                 skip_group_check=True)
            pt_sb = small.tile([P, GROUP_ROWS * n], f16, tag="pt",
                               name=f"pt_sb_{i}")
            nc.vector.tensor_copy(pt_sb[:], pt_ps[:])
            for c in range(n):
                cw = min(P, w - c * P)
                nc.tensor.matmul(
                    out_ps[band, :],
                    pt_sb[:cw, GROUP_ROWS * c: GROUP_ROWS * (c + 1)],
                    vts[(i, c)][:cw, :],
                    start=(c == 0), stop=(c == n - 1),
                    tile_position=(0, GROUP_ROWS * i),
                    skip_group_check=True)

        # ---- head: q projection + group 0, with kT loads for the other
        # groups interleaved between group 0's sub-batches ----
        qp_ps = pmix.tile([P, P], f32, tag="mix", name="qp_ps")
        nc.tensor.matmul(qp_ps[:], wq_sb[:], qt_sb[:], start=True, stop=True)
        qp_sb = const.tile([P, P], f32)
        nc.vector.tensor_copy(qp_sb[:], qp_ps[:])

        emit_load(0)
        emit_mask(0)
        if not l0flags[0]:
            emit_scores(0, [0, 1])
            emit_load(1)
            emit_scores(0, [2])
            emit_load(2)
            emit_load(3)
            emit_scores(0, [3])
        else:
            emit_load(1)
            emit_load(2)
            emit_load(3)
        emit_vdma(0)

        # ---- main loop: group i's softmax+attnV are emitted right after
        # sub-batch 0 of group i+1's tanh stream ----
        for i in range(1, B):
            emit_mask(i)
            if not l0flags[i]:
                emit_scores(i, [0])
                emit_softmax_attnv(i - 1)
                emit_scores(i, range(1, NSB))
            else:
                emit_softmax_attnv(i - 1)
            emit_vdma(i)
        emit_softmax_attnv(B - 1)

        out_sb = const.tile([P, VS], f32)
        nc.vector.tensor_scalar_mul(out_sb[:], out_ps[:], rinv[:, 0:1])
        nc.sync.dma_start(out_d[:], out_sb[:])

    nc.compile()
    return nc


def _get_program(cfg):
    if cfg not in _prog_cache:
        _prog_cache[cfg] = _build_program(cfg)
    return _prog_cache[cfg]


def _width(L):
    # even-padded computed width; valid_len==0 means "uniform over all KV"
    if L <= 0:
        return KV
    L = min(L, KV)
    return min(KV, max(2, 2 * math.ceil(L / 2)))


def kernel(queries, keys, values, valid_lens, Wq, Wk, wv):
    global LAST_EXEC_NS
    queries = np.ascontiguousarray(np.asarray(queries), dtype=np.float32)
    keys = np.ascontiguousarray(np.asarray(keys), dtype=np.float32)
    values = np.ascontiguousarray(np.asarray(values), dtype=np.float32)
    Wq = np.ascontiguousarray(np.asarray(Wq), dtype=np.float32)
    Wk = np.ascontiguousarray(np.asarray(Wk), dtype=np.float32)
    wv = np.ascontiguousarray(np.asarray(wv), dtype=np.float32)
    vl = [int(x) for x in np.asarray(valid_lens)]

    W_b = [_width(L) for L in vl]
    desc = sorted(range(B), key=lambda b: (-W_b[b], b))
    perm = [2, 0, 1, 3]          # fast pipeline start, smallest group last
    gorder = [desc[p] for p in perm]
    Ws = tuple(W_b[b] for b in gorder)
    l0flags = tuple(vl[b] == 0 for b in gorder)
    Wmax = max(Ws)

    nc = _get_program((Ws, l0flags))

    kT = np.concatenate(
        [keys[gorder[i]][:Ws[i]].T for i in range(B)], axis=1)
    kT = np.ascontiguousarray(kT)                        # [128, SW]
    Vm = np.ascontiguousarray(
        np.stack([values[gorder[i]] for i in range(B)]).astype(np.float16))
    # row 0: band indicator x per-group valid mask; row 1: outside-band
    # indicator x MASK_VAL (so non-band score rows exp to exactly 0)
    ind = np.zeros((2, B * P), np.float16)
    for i in range(B):
        ind[0, i * P + GROUP_ROWS * i: i * P + GROUP_ROWS * (i + 1)] = 1.0
        ind[1, i * P: (i + 1) * P] = 1.0
        ind[1, i * P + GROUP_ROWS * i: i * P + GROUP_ROWS * (i + 1)] = 0.0
    mask = np.zeros((2, B * Wmax), np.float16)
    mask[1, :] = MASK_VAL
    for i in range(B):
        L = vl[gorder[i]]
        if L > 0:
            mask[0, i * Wmax + min(L, Ws[i]): i * Wmax + Ws[i]] = MASK_VAL
    wvd = np.zeros((P, 2 * P - 1), np.float16)
    wvd[:, P - 1] = wv.astype(np.float16)
    bident = np.ascontiguousarray(
        np.tile(np.eye(GROUP_ROWS, dtype=np.float16), (B, 1)))

    blobq = np.zeros((P, 2 * P), np.float32)
    blobq[:, P:2 * P] = Wq
    shared = {"wk": np.ascontiguousarray(Wk), "kT": kT, "V": Vm, "ind": ind,
              "mask": mask, "wvd": wvd, "bident": bident}
    in_maps = []
    for c in range(N_CORES):
        qT = np.concatenate(
            [queries[gorder[i], c * GROUP_ROWS:(c + 1) * GROUP_ROWS, :].T
             for i in range(B)], axis=1)
        bl = blobq.copy()
        bl[:, 0:P] = qT
        m = dict(shared)
        m["blobq"] = bl
        in_maps.append(m)

    if SIMULATE:
        from concourse.bass_interp import CoreSim
        outs = []
        for c in range(N_CORES):
            sim = CoreSim(nc, trace=False)
            for name, v in in_maps[c].items():
                sim.tensor(name)[:] = v
            sim.simulate(check_with_hw=False)
            outs.append(sim.tensor("out").copy())
    else:
        from concourse import bass_utils
        kw = {}
        if PROFILE:
            kw = {"trace": True}
        res = bass_utils.run_bass_kernel_spmd(nc, in_maps, list(range(N_CORES)),
                                              **kw)
        if PROFILE:
            LAST_EXEC_NS = res.exec_time_ns
            global LAST_RESULTS
            LAST_RESULTS = res
        outs = [res.results[c]["out"] for c in range(N_CORES)]

    out = np.zeros((B, Q, VS), np.float32)
    for c in range(N_CORES):
        for i in range(B):
            out[gorder[i], c * GROUP_ROWS:(c + 1) * GROUP_ROWS, :] = \
                outs[c][GROUP_ROWS * i: GROUP_ROWS * (i + 1), :]
    return out
